# revision 1
# baseline (speedup 1.0000x reference)
"""XCA (cross-covariance) attention block on 8 TRN2 NeuronCores — v2.

Sharding: 8 cores = 4 batches x 2 image-row-halves (64 rows each).
Per core, per half (32 rows): 1x1 qkv conv (PE, bf16 2col/cyc) -> 3x3
depthwise conv (split: diag-matmul taps on PE + scale/add pairs on
DVE + gpsimd adds) -> PE transposes + gram accumulation -> pairwise
all-reduce of gram+norm partials -> softmax (ln/exp table set only)
-> M = A^T Wp^T (folds attn@v and proj into one GEMM) -> out.
"""

import numpy as np
import ml_dtypes

B, C = 4, 384
HEADS, CHD = 8, 48
WP = 130              # padded row width
HR = 32               # valid rows per half
SP_H = 34 * WP        # 4420 qkv cols per half (32 rows + 2 halo)
DWLEN = 31 * WP + 128  # 4158 dw tap span per half
DWVEC = 4160           # tap vector span (mult of 4 for DVE 4x mode)
DWH = HR * WP         # 4160 dw storage per half
NT = 8192             # valid spatial per core
BF16 = ml_dtypes.bfloat16

# tap assignment: (dy, dx) -> engine. dx=1 taps must be on 'pe' (odd offset).
PE_ALL = [(0, 1), (1, 1), (2, 1), (0, 0), (1, 0), (2, 0)]
QK_PE = [(0, 1), (1, 1), (2, 1), (0, 0), (1, 0)]
QK_VP = [(2, 0), (0, 2), (1, 2), (2, 2)]
QK_GP = []
V_PE = [(0, 1), (1, 1), (2, 1), (0, 0), (1, 0)]
V_VP = [(2, 0), (0, 2), (1, 2), (2, 2)]
V_GP = []
TR_DMA_MOD = 0        # every Nth chunk's transposes via DMA xbar (0 = none)

_CACHE = {}


def _tiles(total, step):
    out = []
    s = 0
    while s < total:
        out.append((s, min(step, total - s)))
        s += step
    return out


def _build_body(nc, tc, tens):
    import concourse.mybir as mybir
    dt = mybir.dt
    Alu = mybir.AluOpType
    Act = mybir.ActivationFunctionType
    AX = mybir.AxisListType

    xd, wqd, dtd, wpd, idd, mkd, tpd, dsd, outd = tens

    cst = tc.alloc_tile_pool(name="cst", bufs=1)
    dwp = tc.alloc_tile_pool(name="dwp", bufs=1)
    wk = tc.alloc_tile_pool(name="wk", bufs=4)
    mmp = tc.alloc_tile_pool(name="mm", bufs=4, space="PSUM")
    drp = tc.alloc_tile_pool(name="dr", bufs=1, space="DRAM")
    xpool = tc.alloc_tile_pool(name="xp", bufs=1)
    qpp = tc.alloc_tile_pool(name="qpp", bufs=2)
    tmp = tc.alloc_tile_pool(name="tmp", bufs=1)

    # ---- constants ----
    wq = cst.tile([128, 9 * 384], dt.bfloat16, tag="wq")
    nc.sync.dma_start(out=wq[:], in_=wqd.ap())
    dtt = cst.tile([128, 54 * 128], dt.bfloat16, tag="dtt")
    nc.sync.dma_start(out=dtt[:], in_=dtd.ap())
    wp_sb = cst.tile([128, 3 * 384], dt.bfloat16, tag="wp")
    nc.sync.dma_start(out=wp_sb[:], in_=wpd.ap())
    identb = cst.tile([128, 128], dt.bfloat16, tag="id")
    nc.sync.dma_start(out=identb[:], in_=idd.ap())
    mask_sb = cst.tile([128, 3 * 384], dt.bfloat16, tag="mk")
    nc.sync.dma_start(out=mask_sb[:], in_=mkd.ap())
    tempc_sb = cst.tile([128, 4], dt.float32, tag="tc")
    nc.sync.dma_start(out=tempc_sb[:], in_=tpd.ap())
    dwsc_sb = cst.tile([128, 81], dt.float32, tag="ds")
    nc.sync.dma_start(out=dwsc_sb[:], in_=dsd.ap())

    cc_sb = cst.tile([128, 1160], dt.bfloat16, tag="cc")
    nc.vector.memset(cc_sb[:, 1158:1160], 0.0)
    nsum = cst.tile([128, 16], dt.float32, tag="ns")
    scr = cst.tile([128, 8], dt.float32, tag="scr")

    # preheat ln/exp act table set (the only set used: sqrt via exp(.5 ln))
    nc.scalar.activation(scr[:, 0:1], tempc_sb[:, 0:1], Act.Ln)
    nc.scalar.activation(scr[:, 1:2], scr[:, 0:1], Act.Exp)
    nc.scalar.activation(scr[:, 2:3], scr[:, 1:2], Act.Square,
                         accum_out=scr[:, 3:4])

    # dw tiles: q g0-2, k g3-5 (per-half, reused across halves);
    # v gv0-2 (full, both halves — needed at stage E)
    dwq = [dwp.tile([128, DWH], dt.bfloat16, tag=f"g{i}", name=f"g{i}")
           for i in range(3)]
    dwk = [dwp.tile([128, DWH], dt.bfloat16, tag=f"g{3+i}", name=f"g{3+i}")
           for i in range(3)]
    dwv = [dwp.tile([128, 2 * DWH], dt.bfloat16, tag=f"gv{i}", name=f"gv{i}")
           for i in range(3)]

    gtp = tc.alloc_tile_pool(name="gt", bufs=1, space="PSUM")
    gt_ps = [gtp.tile([128, 384], dt.float32, tag=f"gt{i}", name=f"gt{i}")
             for i in range(3)]
    tpp = tc.alloc_tile_pool(name="tp", bufs=1, space="PSUM")

    def load_x(h):
        xs = [xpool.tile([128, SP_H], dt.bfloat16, tag=f"x{cb}", name=f"x{cb}h{h}")
              for cb in range(3)]
        for cb in range(3):
            nc.sync.dma_start(
                out=xs[cb][:, 0:2210],
                in_=xd.ap()[cb, :, 32 * h * WP:32 * h * WP + 2210])
        for cb in range(3):
            nc.sync.dma_start(
                out=xs[cb][:, 2210:SP_H],
                in_=xd.ap()[cb, :, 32 * h * WP + 2210:32 * h * WP + SP_H])
        return xs

    pending_norm = []

    def flush_norm():
        while pending_norm:
            pending_norm.pop(0)()

    def block(h, ob, dwt, hoff, xs, is_qk, bidx=None):
        """conv + depthwise for out-block ob; dw result -> dwt[:, hoff:hoff+DWLEN]."""
        pe, vp, gp = (QK_PE, QK_VP, QK_GP) if is_qk else (V_PE, V_VP, V_GP)
        qkv_pre = qpp.tile([128, SP_H + 4], dt.bfloat16, tag="qp")
        nc.gpsimd.memset(qkv_pre[:, SP_H:SP_H + 4], 0.0)
        nc.gpsimd.memset(dwt[:, hoff + DWLEN:hoff + DWVEC], 0.0)
        # 1x1 conv
        for (s, n) in _tiles(SP_H, 512):
            ps = mmp.tile([128, 512], dt.float32, tag="mm")
            for cb in range(3):
                nc.tensor.matmul(
                    ps[:, 0:n],
                    lhsT=wq[:, ob * 384 + cb * 128: ob * 384 + (cb + 1) * 128],
                    rhs=xs[cb][:, s:s + n],
                    start=(cb == 0), stop=(cb == 2))
            nc.scalar.copy(qkv_pre[:, s:s + n], ps[:, 0:n])
        # depthwise: PE diag taps accumulate in psum; scalar copy inits acc
        for (s, n) in _tiles(DWLEN, 512):
            ps = mmp.tile([128, 512], dt.float32, tag="mm")
            for i, (dy, dx) in enumerate(pe):
                off = dy * WP + dx
                t9 = (ob * 6 + PE_ALL.index((dy, dx))) * 128
                nc.tensor.matmul(
                    ps[:, 0:n],
                    lhsT=dtt[:, t9:t9 + 128],
                    rhs=qkv_pre[:, off + s: off + s + n],
                    start=(i == 0), stop=(i == len(pe) - 1))
            nc.scalar.copy(dwt[:, hoff + s: hoff + s + n], ps[:, 0:n])
        # DVE taps: tree-accumulate into temp a (no dependence on the PE
        # partial), then one TT folds a into the accumulator tile.
        tva = tmp.tile([128, DWVEC], dt.bfloat16, tag="tva")
        tvb = tmp.tile([128, DWVEC], dt.bfloat16, tag="tvb")
        for j, (dy, dx) in enumerate(vp):
            off = dy * WP + dx
            k9 = dy * 3 + dx
            dst = tva if j == 0 else tvb
            nc.vector.tensor_scalar_mul(
                dst[:], qkv_pre[:, off:off + DWVEC],
                dwsc_sb[:, ob * 9 + k9:ob * 9 + k9 + 1])
            if j > 0:
                nc.vector.tensor_tensor(out=tva[:], in0=tva[:], in1=tvb[:],
                                        op=Alu.add)
        nc.vector.tensor_tensor(
            out=dwt[:, hoff:hoff + DWVEC], in0=dwt[:, hoff:hoff + DWVEC],
            in1=tva[:], op=Alu.add)
        # norms (sum of squares over valid window), deferred one block so the
        # Square doesn't block the next block's copies in the scalar queue;
        # scratch reuses this block's qkv_pre buffer (dead after the last tap)
        if is_qk:
            v = dwt[:, hoff:hoff + DWH].rearrange("p (r c) -> p r c", c=WP)

            def norm_fn(v=v, qkv_pre=qkv_pre, bidx=bidx, h=h):
                nc.scalar.activation(
                    qkv_pre[:, 0:4096].rearrange("p (r c) -> p r c", c=128),
                    v[:, :, 0:128], Act.Square,
                    accum_out=nsum[:, bidx * 2 + h: bidx * 2 + h + 1])
            pending_norm.append(norm_fn)


    def transposes_and_gram(h, r0=0, r1=HR):
        flush_norm()
        for r in range(r0, r1):
            qt = wk.tile([128, 384], dt.bfloat16, tag="qt")
            kt = wk.tile([128, 384], dt.bfloat16, tag="kt")
            if TR_DMA_MOD and (r % TR_DMA_MOD) == (TR_DMA_MOD - 1):
                for i in range(3):
                    nc.sync.dma_start_transpose(
                        qt[:, i * 128:(i + 1) * 128], dwq[i][:, r * WP:r * WP + 128])
                    nc.sync.dma_start_transpose(
                        kt[:, i * 128:(i + 1) * 128], dwk[i][:, r * WP:r * WP + 128])
            else:
                tq = tpp.tile([128, 768], dt.bfloat16, tag="tq")
                for i in range(3):
                    nc.tensor.transpose(
                        tq[:, i * 128:(i + 1) * 128],
                        dwq[i][:, r * WP:r * WP + 128].opt(), identb[:])
                    nc.tensor.transpose(
                        tq[:, 384 + i * 128: 384 + (i + 1) * 128],
                        dwk[i][:, r * WP:r * WP + 128].opt(), identb[:])
                if r % 2 == 0:
                    nc.scalar.copy(qt[:], tq[:, 0:384])
                    nc.scalar.copy(kt[:], tq[:, 384:768])
                else:
                    nc.vector.tensor_copy(qt[:], tq[:, 0:384])
                    nc.vector.tensor_copy(kt[:], tq[:, 384:768])
            first = (r == 0)
            last = (r == HR - 1)
            for i in range(3):
                nc.tensor.matmul(
                    gt_ps[i][:], lhsT=kt[:, i * 128:(i + 1) * 128], rhs=qt[:],
                    start=first, stop=last)

    # ---------------- phase A/B: TG segments interleave with v blocks ----
    xs = load_x(0)
    for i in range(3):
        block(0, i, dwq[i], 0, xs, True, bidx=i)
        block(0, 3 + i, dwk[i], 0, xs, True, bidx=3 + i)
    block(0, 6, dwv[0], 0, xs, False)
    transposes_and_gram(0, 0, 11)
    block(0, 7, dwv[1], 0, xs, False)
    transposes_and_gram(0, 11, 22)
    block(0, 8, dwv[2], 0, xs, False)
    transposes_and_gram(0, 22, 32)
    # ---- collective #1: h0 gram + h0 norms, overlaps all of h1 ----
    flush_norm()
    for b6 in range(6):
        nc.vector.tensor_copy(cc_sb[:, 1152 + b6:1153 + b6],
                              nsum[:, b6 * 2:b6 * 2 + 1])
    for i in range(3):
        nc.vector.tensor_copy(cc_sb[:, 384 * i:384 * (i + 1)], gt_ps[i][:])
    cc1_in = drp.tile([128, 1160], dt.bfloat16, tag="cc1i")
    cc1_out = drp.tile([128, 1160], dt.bfloat16, tag="cc1o")
    nc.gpsimd.dma_start(out=cc1_in[:], in_=cc_sb[:])
    with nc.allow_low_precision(reason="gram/norm partials all-reduced in bf16"):
        nc.gpsimd.collective_compute(
            "AllReduce", Alu.add,
            replica_groups=[[0, 1], [2, 3], [4, 5], [6, 7]],
            ins=[cc1_in.opt()], outs=[cc1_out.opt()])
    g1_sb = cst.tile([128, 1160], dt.bfloat16, tag="g1")
    nc.gpsimd.dma_start(out=g1_sb[:], in_=cc1_out[:])
    xs1 = load_x(1)
    for i in range(3):
        block(1, i, dwq[i], 0, xs1, True, bidx=i)
        block(1, 3 + i, dwk[i], 0, xs1, True, bidx=3 + i)
    block(1, 6, dwv[0], DWH, xs1, False)
    transposes_and_gram(1, 0, 16)
    transposes_and_gram(1, 16, 32)

    # norm partials + gram (h1) -> cc buffer
    flush_norm()
    for b6 in range(6):
        nc.vector.tensor_copy(cc_sb[:, 1152 + b6:1153 + b6],
                              nsum[:, b6 * 2 + 1:b6 * 2 + 2])
    for i in range(3):
        nc.vector.tensor_copy(cc_sb[:, 384 * i:384 * (i + 1)], gt_ps[i][:])
    tpp.release()
    gtp.release()

    # ---------------- collective (pairwise all-reduce) ----------------
    cc_in = drp.tile([128, 1160], dt.bfloat16, tag="ccin")
    cc_out = drp.tile([128, 1160], dt.bfloat16, tag="ccout")
    nc.gpsimd.dma_start(out=cc_in[:], in_=cc_sb[:])
    with nc.allow_low_precision(reason="gram/norm partials all-reduced in bf16"):
        nc.gpsimd.collective_compute(
            "AllReduce", Alu.add,
            replica_groups=[[0, 1], [2, 3], [4, 5], [6, 7]],
            ins=[cc_in.opt()], outs=[cc_out.opt()])
    # last h1 v blocks fill the collective latency window
    block(1, 7, dwv[1], DWH, xs1, False)
    block(1, 8, dwv[2], DWH, xs1, False)
    tmp.release()
    qpp.release()
    xpool.release()
    flush_norm()
    smp = tc.alloc_tile_pool(name="smp", bufs=2)
    g_sb = cc_sb
    nc.gpsimd.dma_start(out=g_sb[:], in_=cc_out[:])
    nc.vector.tensor_tensor(out=g_sb[:, 0:1160], in0=g_sb[:, 0:1160],
                            in1=g1_sb[:], op=Alu.add)

    # ---------------- softmax ----------------
    # rt = sqrt(ss) via exp(0.5*ln(ss)) (keeps single act table set)
    rt = cst.tile([128, 8], dt.float32, tag="rt")
    nc.scalar.activation(rt[:, 0:6], g_sb[:, 1152:1158], Act.Ln)
    nc.scalar.activation(rt[:, 0:6], rt[:, 0:6], Act.Exp, scale=0.5)
    nc.vector.tensor_scalar_max(rt[:, 0:6], rt[:, 0:6], 1e-12)
    inv = cst.tile([128, 8], dt.float32, tag="inv")
    nc.vector.reciprocal(inv[:, 0:6], rt[:, 0:6])

    gts_t = [cst.tile([128, 384], dt.bfloat16, tag=f"gs{i}", name=f"gsb{i}")
             for i in range(3)]
    for i in range(3):  # scale G^T rows (k channels, block i) by inv_k
        nc.vector.tensor_scalar_mul(
            gts_t[i][:], g_sb[:, 384 * i:384 * (i + 1)], inv[:, 3 + i:4 + i])
    gsp = tc.alloc_tile_pool(name="gs", bufs=1, space="PSUM")
    gs_ps = [gsp.tile([128, 384], dt.bfloat16, tag=f"gp{j}", name=f"gsp{j}")
             for j in range(3)]
    for j in range(3):
        for i in range(3):
            nc.tensor.transpose(
                gs_ps[j][:, i * 128:(i + 1) * 128],
                gts_t[i][:, j * 128:(j + 1) * 128], identb[:])
    attn_t = [cst.tile([128, 384], dt.bfloat16, tag=f"at{j}", name=f"attn{j}")
              for j in range(3)]
    sums = cst.tile([128, 4], dt.float32, tag="sm")
    for j in range(3):
        zf = smp.tile([128, 384], dt.float32, tag="zf")
        nc.vector.tensor_scalar_mul(zf[:], gs_ps[j][:], inv[:, j:j + 1])
        z = smp.tile([128, 384], dt.float32, tag="zz")
        nc.vector.scalar_tensor_tensor(
            z[:], zf[:], tempc_sb[:, j:j + 1], mask_sb[:, 384 * j:384 * (j + 1)],
            op0=Alu.mult, op1=Alu.add)
        negmax = smp.tile([128, 1], dt.float32, tag="nm")
        nc.vector.tensor_reduce(negmax[:], z[:], AX.X, Alu.max, negate=True)
        nc.scalar.activation(
            attn_t[j][:], z[:], Act.Exp, bias=negmax[:, 0:1],
            accum_out=sums[:, j:j + 1])
    gsp.release()
    invs = cst.tile([128, 4], dt.float32, tag="is")
    nc.vector.reciprocal(invs[:, 0:3], sums[:, 0:3])
    for j in range(3):
        nc.vector.tensor_scalar_mul(attn_t[j][:], attn_t[j][:], invs[:, j:j + 1])

    # keep PE warm through the softmax serial section
    warm = tc.alloc_tile_pool(name="warm", bufs=1, space="PSUM")
    wps = warm.tile([128, 384], dt.float32, tag="wm")
    nc.tensor.matmul(wps[:, 0:128], lhsT=g_sb[:, 0:128], rhs=wp_sb[:, 0:128],
                     start=True, stop=True)
    for j in range(3):
        nc.tensor.matmul(wps[:, 0:128], lhsT=gts_t[j][:, 0:128],
                         rhs=wp_sb[:, 0:128], start=True, stop=True)
        nc.tensor.matmul(wps[:, 0:128], lhsT=attn_t[j][:, 0:128],
                         rhs=wp_sb[:, 0:128], start=True, stop=True)
    warm.release()

    # ---------------- M = A^T Wp^T  [d, o] ----------------
    mp = tc.alloc_tile_pool(name="mp", bufs=1, space="PSUM")
    m_ps = [mp.tile([128, 384], dt.float32, tag=f"m{i}", name=f"mps{i}")
            for i in range(3)]
    for db in range(3):
        cbs = [cb for cb in range(3) if abs(cb - db) <= 1]
        for idx, cb in enumerate(cbs):
            nc.tensor.matmul(
                m_ps[db][:], lhsT=attn_t[cb][:, db * 128:(db + 1) * 128],
                rhs=wp_sb[:, cb * 384:(cb + 1) * 384],
                start=(idx == 0), stop=(idx == len(cbs) - 1))
    m_sb = cst.tile([128, 3 * 384], dt.bfloat16, tag="msb")
    for db in range(3):
        nc.scalar.copy(m_sb[:, db * 384:(db + 1) * 384], m_ps[db][:])
    mp.release()

    # ---------------- out = M^T @ V  + store ----------------
    otp = tc.alloc_tile_pool(name="otp", bufs=2)
    for ob in range(3):
        for t in range(16):
            ps = mmp.tile([128, 512], dt.float32, tag="mm")
            hh, r4 = t // 8, (t % 8) * 4
            for db in range(3):
                vv = dwv[db][:, hh * DWH:(hh + 1) * DWH].rearrange(
                    "p (r c) -> p r c", c=WP)
                nc.tensor.matmul(
                    ps[:],
                    lhsT=m_sb[:, db * 384 + ob * 128: db * 384 + ob * 128 + 128],
                    rhs=vv[:, r4:r4 + 4, 0:128],
                    start=(db == 0), stop=(db == 2))
            ot = otp.tile([128, 512], dt.float32, tag="ot")
            if t % 2 == 0:
                nc.scalar.copy(ot[:], ps[:])
            else:
                nc.vector.tensor_copy(ot[:], ps[:])
            nc.sync.dma_start(
                out=outd.ap()[ob, :, 512 * t:512 * (t + 1)], in_=ot[:])

    for p in (otp, smp, drp, mmp, wk, dwp, cst):
        p.release()


def build_nc():
    if "nc" in _CACHE:
        return _CACHE["nc"]
    from concourse import bacc, tile
    import concourse.mybir as mybir
    dt = mybir.dt
    nc = bacc.Bacc("TRN2", target_bir_lowering=False, debug=False, num_devices=8)
    xd = nc.dram_tensor("x", [3, 128, 66 * WP], dt.bfloat16, kind="ExternalInput")
    wqd = nc.dram_tensor("wq", [128, 9 * 384], dt.bfloat16, kind="ExternalInput")
    dtd = nc.dram_tensor("dtap", [128, 54 * 128], dt.bfloat16,
                         kind="ExternalInput")
    wpd = nc.dram_tensor("wp", [128, 3 * 384], dt.bfloat16, kind="ExternalInput")
    idd = nc.dram_tensor("identb", [128, 128], dt.bfloat16, kind="ExternalInput")
    mkd = nc.dram_tensor("maskt", [128, 3 * 384], dt.bfloat16, kind="ExternalInput")
    tpd = nc.dram_tensor("tempc", [128, 4], dt.float32, kind="ExternalInput")
    dsd = nc.dram_tensor("dwsc", [128, 81], dt.float32, kind="ExternalInput")
    outd = nc.dram_tensor("out", [3, 128, NT], dt.float32, kind="ExternalOutput")
    with tile.TileContext(nc) as tc:
        _build_body(nc, tc, (xd, wqd, dtd, wpd, idd, mkd, tpd, dsd, outd))
    nc.compile()
    _CACHE["nc"] = nc
    return nc


def make_in_maps(x, qkv_w, dw_w, proj_w, temperature):
    x = np.asarray(x, np.float32)
    qkv_w = np.asarray(qkv_w, np.float32)
    dw_w = np.asarray(dw_w, np.float32)
    proj_w = np.asarray(proj_w, np.float32)
    temperature = np.asarray(temperature, np.float32).reshape(-1)

    xp = np.zeros((B, C, 130, 130), np.float32)
    xp[:, :, 1:129, 1:129] = x

    wq = np.zeros((128, 9 * 384), np.float32)
    for ob in range(9):
        for cb in range(3):
            blk = qkv_w[ob * 128:(ob + 1) * 128, cb * 128:(cb + 1) * 128]
            wq[:, ob * 384 + cb * 128: ob * 384 + (cb + 1) * 128] = blk.T
    # diag tap matrices: one per (ob, pe-tap) in QK_PE order
    dtap = np.zeros((128, 54 * 128), np.float32)
    rng = np.arange(128)
    for ob in range(9):
        for ti, (dy, dx) in enumerate(PE_ALL):
            col = (ob * 6 + ti) * 128
            dtap[rng, col + rng] = dw_w[ob * 128 + rng, 0, dy, dx]
    wpm = np.zeros((128, 3 * 384), np.float32)
    for cb in range(3):
        wpm[:, cb * 384:(cb + 1) * 384] = proj_w[:, cb * 128:(cb + 1) * 128].T
    ident = np.eye(128, dtype=np.float32)
    mask = np.full((128, 3 * 384), -1e30, np.float32)
    for j in range(3):
        for p in range(128):
            hgrp = (128 * j + p) // CHD
            mask[p, 384 * j + CHD * hgrp: 384 * j + CHD * (hgrp + 1)] = 0.0
    tempc = np.zeros((128, 4), np.float32)
    for j in range(3):
        for p in range(128):
            tempc[p, j] = temperature[(128 * j + p) // CHD]
    dwsc = np.zeros((128, 81), np.float32)
    for ob in range(9):
        for k9 in range(9):
            dwsc[:, ob * 9 + k9] = dw_w[ob * 128:(ob + 1) * 128, 0, k9 // 3, k9 % 3]

    shared = {
        "wq": wq.astype(BF16), "dtap": dtap.astype(BF16),
        "wp": wpm.astype(BF16), "identb": ident.astype(BF16),
        "maskt": mask.astype(BF16), "tempc": tempc, "dwsc": dwsc,
    }
    in_maps = []
    for core in range(8):
        b, s = core // 2, core % 2
        xs = xp[b, :, 64 * s: 64 * s + 66, :]
        xs = np.ascontiguousarray(xs.reshape(3, 128, 66 * WP)).astype(BF16)
        m = dict(shared)
        m["x"] = xs
        in_maps.append(m)
    return in_maps


def assemble(results):
    full = np.zeros((B, C, 128, 128), np.float32)
    for core in range(8):
        b, s = core // 2, core % 2
        o = np.asarray(results[core]["out"], np.float32).reshape(C, 64, 128)
        full[b, :, 64 * s: 64 * s + 64, :] = o
    return full


def kernel(x, qkv_w, dw_w, proj_w, temperature):
    from concourse.bass_utils import run_bass_kernel_spmd
    nc = build_nc()
    in_maps = make_in_maps(x, qkv_w, dw_w, proj_w, temperature)
    res = run_bass_kernel_spmd(nc, in_maps, core_ids=list(range(8)))
    return assemble(res.results)

